# revision 11
# baseline (speedup 1.0000x reference)
"""4-bit column-block-quantized linear (ColBlockQuantizedLinear) on 8 TRN2 cores.

Math:  out[b,o] = scales[o] * (sum_i inp[b,i]*wq[o,i] - zeros[o]*rowsum[b])
where wq comes from packed bytes q[o,j] (j = i//2): even i -> low nibble,
odd i -> high nibble.

Device-side identity (keeps all O(O*I) work on-device with 2 elementwise
ops per byte tile):
    sum_j l[j,o]*a[j] + sum_j h[j,o]*b[j]  =  sum_j q[j,o]*a[j] + sum_j h[j,o]*(b[j]-16a[j])
with q = 16h + l, a[j]=inp[:,2j], b[j]=inp[:,2j+1].  q (0..255) and h (0..15)
are exact in bf16, so the only elementwise device ops are a cast (ACT) and a
right-shift (DVE).  Activations are hi/lo bf16-split (stationary [128,32]) so
the bf16 matmuls give ~fp32 accuracy; the zeros*rowsum rank-1 term is a K=4
matmul with hi/lo-split factors.

Sharding: column-parallel over out_features (1376 rows/core), inputs
replicated; per-core output [16,1376] gathered on host.
"""

import numpy as np
import ml_dtypes

B = 16
I = 4096
O = 11008
NCORES = 8
OS = O // NCORES          # 1376 out-features per core
HALF = I // 2             # 2048 packed columns
KT = HALF // 128          # 16 contraction tiles
BLKS = [(0, 512), (512, 512), (1024, OS - 1024)]  # psum-bank-sized o-blocks

BF16 = ml_dtypes.bfloat16

_CACHE = {}


def _split_hi_lo(x64):
    """Split float64 array into (hi, lo) bf16 parts: hi+lo ~= x to ~2^-17."""
    hi = x64.astype(BF16)
    lo = (x64 - hi.astype(np.float64)).astype(BF16)
    return hi, lo


def _build_program():
    import concourse.bacc as bacc
    import concourse.mybir as mybir
    import concourse.tile as tile

    dt = mybir.dt
    nc = bacc.Bacc("TRN2", target_bir_lowering=False)

    q = nc.dram_tensor("q", [HALF, OS], dt.uint8, kind="ExternalInput")
    statA = nc.dram_tensor("statA", [128, KT * 64], dt.bfloat16, kind="ExternalInput")
    statC = nc.dram_tensor("statC", [128, KT * 64], dt.bfloat16, kind="ExternalInput")
    corrL = nc.dram_tensor("corrL", [4, 64], dt.bfloat16, kind="ExternalInput")
    corrR = nc.dram_tensor("corrR", [4, OS], dt.bfloat16, kind="ExternalInput")
    sc = nc.dram_tensor("sc", [B, OS], dt.float32, kind="ExternalInput")
    out = nc.dram_tensor("out", [B, OS], dt.float32, kind="ExternalOutput")

    with tile.TileContext(nc) as tc:
        with (
            tc.tile_pool(name="consts", bufs=1) as cpool,
            tc.tile_pool(name="qp", bufs=3) as qpool,
            tc.tile_pool(name="wp", bufs=3) as wpool,
            tc.tile_pool(name="op", bufs=2) as opool,
            tc.tile_pool(name="ps", bufs=1, space="PSUM") as pspool,
        ):
            statA_sb = cpool.tile([128, KT * 64], dt.bfloat16, name="statA_sb")
            statC_sb = cpool.tile([128, KT * 64], dt.bfloat16, name="statC_sb")
            corrL_sb = cpool.tile([4, 64], dt.bfloat16, name="corrL_sb")
            corrR_sb = cpool.tile([4, OS], dt.bfloat16, name="corrR_sb")
            sc_sb = cpool.tile([B, OS], dt.float32, name="sc_sb")
            nc.sync.dma_start(statA_sb, statA[:, :])
            nc.sync.dma_start(statC_sb, statC[:, :])
            nc.sync.dma_start(corrL_sb, corrL[:, :])
            nc.sync.dma_start(corrR_sb, corrR[:, :])
            nc.sync.dma_start(sc_sb, sc[:, :])

            psums = [
                pspool.tile([64, n], dt.float32, name=f"ps{i}")
                for i, (s, n) in enumerate(BLKS)
            ]

            for kt in range(KT):
                qt = qpool.tile([128, OS], dt.uint8, name="qt", tag="qt")
                nc.gpsimd.dma_start(qt, q[kt * 128 : (kt + 1) * 128, :])
                qb = wpool.tile([128, OS], dt.bfloat16, name="qb", tag="qb")
                ht = qpool.tile([128, OS], dt.uint8, name="ht", tag="ht")
                hb = wpool.tile([128, OS], dt.bfloat16, name="hb", tag="hb")
                # cast bytes 0..255 to bf16 (exact) on ACT
                nc.scalar.activation(qb, qt, mybir.ActivationFunctionType.Copy)
                # high nibble: u8 shift on DVE, then cast u8->bf16
                nc.vector.tensor_scalar(
                    ht, qt, 4, None, mybir.AluOpType.logical_shift_right
                )
                nc.vector.tensor_copy(hb, ht)
                for i, (s, n) in enumerate(BLKS):
                    nc.tensor.matmul(
                        psums[i],
                        statA_sb[:, kt * 64 : kt * 64 + 64],
                        qb[:, s : s + n],
                        start=(kt == 0),
                        stop=False,
                    )
                    nc.tensor.matmul(
                        psums[i],
                        statC_sb[:, kt * 64 : kt * 64 + 64],
                        hb[:, s : s + n],
                        start=False,
                        stop=False,
                    )

            for i, (s, n) in enumerate(BLKS):
                # rank-1 zeros*rowsum correction (cols 16:32 of corrL are 0)
                nc.tensor.matmul(
                    psums[i],
                    corrL_sb,
                    corrR_sb[:, s : s + n],
                    start=False,
                    stop=True,
                )
                t0 = opool.tile([B, n], dt.float32, name="t0", tag=f"t0{i}")
                t = opool.tile([B, n], dt.float32, name="t", tag=f"t{i}")
                o = opool.tile([B, n], dt.float32, name="o", tag=f"o{i}")
                # lo-group psum -> sbuf on ACT (only one psum read allowed per TT)
                nc.scalar.activation(
                    t0, psums[i][32:48, :], mybir.ActivationFunctionType.Copy
                )
                nc.vector.tensor_tensor(
                    t, psums[i][0:16, :], t0, mybir.AluOpType.add
                )
                nc.vector.tensor_tensor(
                    o, t, sc_sb[:, s : s + n], mybir.AluOpType.mult
                )
                nc.sync.dma_start(out[:, s : s + n], o)

    nc.finalize()
    return nc


def _get_program():
    if "nc" not in _CACHE:
        _CACHE["nc"] = _build_program()
    return _CACHE["nc"]


def _host_prep(inp, quant_weight, scales, zeros):
    """Build per-core input maps (layout/precision prep only, no dequant math)."""
    inp64 = np.asarray(inp, dtype=np.float64)
    a = inp64[:, 0::2].T.copy()  # [HALF, B] even-i activations (pair with l)
    b = inp64[:, 1::2].T.copy()  # [HALF, B] odd-i activations (pair with h)
    # identity: sum l*a + sum h*b = sum q*a + sum h*(b - 16*a)
    a_hi, a_lo = _split_hi_lo(a)          # stationary paired with cast(q)
    c_hi, c_lo = _split_hi_lo(b - 16.0 * a)  # stationary paired with (q >> 4)

    statA = np.zeros((128, KT * 64), dtype=BF16)
    statC = np.zeros((128, KT * 64), dtype=BF16)
    for kt in range(KT):
        rows = slice(kt * 128, (kt + 1) * 128)
        statA[:, kt * 64 : kt * 64 + 16] = a_hi[rows]
        statA[:, kt * 64 + 32 : kt * 64 + 48] = a_lo[rows]
        statC[:, kt * 64 : kt * 64 + 16] = c_hi[rows]
        statC[:, kt * 64 + 32 : kt * 64 + 48] = c_lo[rows]

    rowsum = inp64.sum(axis=1)  # [B]
    rs_hi, rs_lo = _split_hi_lo(rowsum)
    corrL = np.zeros((4, 64), dtype=BF16)
    corrL[0, :16] = rs_hi
    corrL[1, :16] = rs_hi
    corrL[2, :16] = rs_lo
    corrL[3, :16] = rs_lo

    qw = np.asarray(quant_weight)
    scales = np.asarray(scales, dtype=np.float64).reshape(-1)
    zeros = np.asarray(zeros, dtype=np.float64).reshape(-1)

    in_maps = []
    for cidx in range(NCORES):
        rows = slice(cidx * OS, (cidx + 1) * OS)
        qc = np.ascontiguousarray(qw[rows].astype(np.uint8).T)  # [HALF, OS]
        z = zeros[rows]
        z_hi, z_lo = _split_hi_lo(z)
        corrR = np.zeros((4, OS), dtype=BF16)
        corrR[0] = -z_hi
        corrR[1] = -z_lo
        corrR[2] = -z_hi
        corrR[3] = -z_lo
        sc_c = np.broadcast_to(
            scales[rows].astype(np.float32), (B, OS)
        ).copy()
        in_maps.append(
            {
                "q": qc,
                "statA": statA,
                "statC": statC,
                "corrL": corrL,
                "corrR": corrR,
                "sc": sc_c,
            }
        )
    return in_maps


def kernel(inp, quant_weight, scales, zeros):
    from concourse.bass_utils import run_bass_kernel_spmd

    nc = _get_program()
    in_maps = _host_prep(inp, quant_weight, scales, zeros)
    res = run_bass_kernel_spmd(nc, in_maps, core_ids=list(range(NCORES)))
    out = np.concatenate(
        [res.results[c]["out"] for c in range(NCORES)], axis=1
    )
    return np.ascontiguousarray(out.astype(np.float32))
